# revision 2
# baseline (speedup 1.0000x reference)
"""DirectionalConv3d Trainium2 kernel (v3: full-width DMAs, streamed output).

out[b, o, t, r, c] = sum_d W_d[o, :] . x[b, :, (t,r,c)+delta_d]
for the 7-point directional stencil (self, t+-1, r+-1, c+-1), zero padded.

Strategy (per core, 1 batch per core, 8 cores):
  - Host pre-casts x to bf16 and pads each row to pitch 34 (2 zero cols), so
    c+-1 shifted reads hit zeros at row boundaries with no on-chip restaging.
  - T-halved image: partitions 0-63 hold planes 0-16, partitions 64-127 hold
    planes 15-31.  Each image chunk is ONE 128-partition DMA with a 2-level
    partition access pattern (half x channel) so descriptors fan out over all
    16 SDMA engines -- two 64-partition DMAs serialize at half bandwidth.
  - No zero halo slots: the t-boundary matmuls (tp on plane 0, tm on plane
    31) contribute exactly zero and are skipped, so no big memsets and no
    zero planes in SBUF.
  - 4 concurrent PE tile-position streams: quadrant (lhs_half*64,
    psum_half*64) computes one output plane per round; rounds r=0..7 cover
    planes (2r, 2r+1, 16+2r, 17+2r).  Each plane = 7 directional matmuls x
    2 psum chunks (out rows 0-15 / 16-31, 512 f32 = 1 bank each).  r+-1
    shifts clip the out-row range instead of padding (skipped rows == +0).
  - d-loop order alternates per round (self..cm / cm..self) so the boundary
    direction's stationaries stay loaded; LDWEIGHTS dedup drops reloads.
  - Evac round r: DVE casts ptile[h=0] [128,1024] f32->bf16 into the stage,
    ACT casts ptile[h=1]; then ONE 128-partition DMA on the scalar HWDGE
    ring (separate from the sync-ring input stream) writes both planes of
    both halves straight to HBM every round, overlapping the input stream.
"""

import numpy as np
import ml_dtypes

B = 8
CI = 64
CO = 64
T = 32
R = 32
C = 32
RP = 34                  # padded row pitch (2 zero cols)
PL = R * RP              # 1088 elems per padded plane (input image)
OPL = R * C              # 1024 elems per dense output plane
N = T * OPL              # dense output elems per channel
LEAD = 2                 # zero guard elems at each image-chunk tile front
HALF = 16                # output planes per partition half
# planes per half per chunk (17 per half incl. the shared halo plane 15/16);
# small first chunks so round 0 compute starts as early as possible
CHUNK_PLANES = (2, 2, 3, 3, 3, 4)

# direction -> (dt, dr, dc); order fixed: self first (start), cm last (stop)
DIRS = [
    ("self", 0, 0, 0),
    ("rp", 0, -1, 0),
    ("rm", 0, 1, 0),
    ("cp", 0, 0, -1),
    ("tp", -1, 0, 0),
    ("tm", 1, 0, 0),
    ("cm", 0, 0, 1),
]
NW = len(DIRS)

_NC_CACHE = {}


def _chunk_of_plane(k):
    c0 = 0
    for ci, n in enumerate(CHUNK_PLANES):
        if c0 <= k < c0 + n:
            return ci, c0, n
        c0 += n
    raise AssertionError(k)


def _emit(nc, tc, x, wt, out, mybir, bass):
    bf16 = mybir.dt.bfloat16
    AP = bass.AP

    xpool = tc.alloc_tile_pool(name="xin", bufs=1)
    wpool = tc.alloc_tile_pool(name="wp", bufs=1)
    apool = tc.alloc_tile_pool(name="accp", bufs=2, space="PSUM")
    spool = tc.alloc_tile_pool(name="stg", bufs=3)

    # ---- weights: host ships [128, 7*64] (both halves pre-stacked) ----
    w_sb = wpool.tile([128, NW * CO], bf16, name="w_sb")
    nc.sync.dma_start(
        out=w_sb[0:128, :],
        in_=AP(wt.tensor, 0, [[NW * CO, 128], [1, NW * CO]]))

    # ---- image chunks: ONE 128-partition DMA each ----
    # top half (parts 0-63) plane k at slot k-c0; bottom half (parts 64-127)
    # plane 15+k at the same column offset: 2-level partition AP with the
    # halves 15 planes apart in HBM.
    xts = []
    c0 = 0
    for ci, npl in enumerate(CHUNK_PLANES):
        # +2 tail: the rearrange view of a cm-shifted read of the last row
        # spans (but never reads) up to 2 elems past the last plane
        xt = xpool.tile([128, LEAD + npl * PL + 2], bf16, name=f"xc{ci}")
        nc.vector.memset(xt[0:128, 0:LEAD], 0.0)
        src = AP(x.tensor, c0 * PL,
                 [[15 * PL, 2], [T * PL, CI], [1, npl * PL]])
        nc.sync.dma_start(out=xt[0:128, LEAD:LEAD + npl * PL], in_=src)
        xts.append(xt)
        c0 += npl

    def w_ap(di, h):
        return w_sb[h * 64:(h + 1) * 64, di * CO:(di + 1) * CO]

    def rhs_ap(h, k, xrow0, nrows, dc):
        """rhs AP: x rows xrow0.., cols dc.. of chunk-slot k of half h."""
        ci, cs0, npl = _chunk_of_plane(k)
        xt = xts[ci]
        lo = h * 64
        start = LEAD + (k - cs0) * PL + xrow0 * RP + dc
        v = xt[lo:lo + 64, start:start + nrows * RP]
        v = v.rearrange("p (r c) -> p r c", c=RP)
        return v[:, :, 0:C]

    # ---- main loop: 8 rounds x 7 dirs x 2 row-chunks x 4 quadrants ----
    for r in range(8):
        # psum tiles: one 2-bank tile per x-half; top/bottom psum partition
        # halves hold the even/odd output plane of this round
        ptiles = {}
        for h in range(2):
            ptiles[h] = apool.tile(
                [128, 1024], mybir.dt.float32, name=f"ps{r}_{h}",
                tag=f"ps{h}")

        # alternate direction order per round: the boundary direction's
        # stationary stays loaded, saving 4 LDWEIGHTS per round boundary
        order = range(NW) if r % 2 == 0 else range(NW - 1, -1, -1)
        for kk, di in enumerate(order):
            dname, dt_, dr, dc = DIRS[di]
            first = kk == 0
            last = kk == NW - 1
            for c2 in range(2):
                row0 = c2 * 16
                # out rows valid iff 0 <= row+dr < R
                orow0 = max(row0, -dr)
                orow1 = min(row0 + 16, R - dr)
                for h in range(2):          # x-half = lhsT partition half
                    for pb in range(2):     # psum partition half
                        po = 2 * r + pb     # local out plane 0..15
                        # slot index: top plane po+dt_, bottom 15+(po+dt_+1)
                        if h == 0:
                            slot = po + dt_
                            if slot < 0:
                                continue    # tp on plane 0: zero contrib
                        else:
                            slot = po + dt_ + 1
                            if slot > 16:
                                continue    # tm on plane 31: zero contrib
                        lo = pb * 64
                        acc = ptiles[h]
                        cb = c2 * 512       # bank base inside 2-bank tile
                        if first or last:   # dr == 0: full bank
                            oap = acc[lo:lo + 64, cb:cb + 512]
                            rhs = rhs_ap(h, slot, row0, 16, dc)
                        else:
                            oap = acc[lo:lo + 64,
                                      cb + (orow0 - row0) * C:
                                      cb + (orow1 - row0) * C]
                            rhs = rhs_ap(h, slot, orow0 + dr,
                                         orow1 - orow0, dc)
                        nc.tensor.matmul(
                            out=oap, lhsT=w_ap(di, h), rhs=rhs,
                            start=first, stop=last,
                            # sim psum-group tracker aliases partition
                            # ranges within a bank; per-element on HW
                            skip_group_check=True)

        # ---- evac round r: 2 full-width cast-copies + 1 out-DMA ----
        stage = spool.tile([128, 2 * OPL], bf16, name=f"st{r}", tag="st")
        nc.vector.tensor_copy(out=stage[:, 0:OPL], in_=ptiles[0][:, :])
        nc.scalar.copy(out=stage[:, OPL:2 * OPL], in_=ptiles[1][:, :])
        # one 128-partition DMA: partition p = pb*64+ch -> channel ch,
        # plane h*16 + 2r + pb; rides the scalar (ACT) HWDGE ring so it
        # never queues behind the sync-ring input stream
        dst = AP(out.tensor, 2 * r * OPL,
                 [[OPL, 2], [N, 64], [HALF * OPL, 2], [1, OPL]])
        nc.scalar.dma_start(out=dst, in_=stage[0:128, :])

    for p in (spool, apool, wpool, xpool):
        p.release()


def _dedup_ldweights(nc, mybir):
    """Drop InstLdweights whose tile position already holds the same
    stationary (the PE keeps per-quadrant weights until overwritten).
    Any semaphore waits on a dropped load are preserved on an InstNoOp in
    its place on the PE queue."""
    SyncInfo = mybir.SyncInfo
    counter = [0]
    last_w = {}
    for f in nc.m.functions:
        for blk in f.blocks:
            out, changed = [], False
            for inst in blk.instructions:
                if type(inst).__name__ != "InstLdweights":
                    out.append(inst)
                    continue
                pos = tuple(inst.tile_position or (0, 0))
                wap = inst.ins[0]
                key = (wap.memref, wap.offset,
                       tuple(tuple(p) for p in wap.ap))
                si = getattr(inst, "sync_info", None)
                ups = list(si.on_update) if si is not None and si.on_update \
                    else []
                if last_w.get(pos) == key and not ups:
                    waits = list(si.on_wait) if si is not None and \
                        si.on_wait else []
                    if waits:
                        nop = mybir.InstNoOp(name=f"ldwnop_{counter[0]}")
                        counter[0] += 1
                        nop.engine = inst.engine
                        nop.sync_info = SyncInfo(on_wait=waits, on_update=[])
                        nc.register_instruction(nop, overwrite=True)
                        out.append(nop)
                    changed = True
                    continue
                last_w[pos] = key
                out.append(inst)
            if changed:
                blk.instructions = out


def _split_multi_waits(nc, mybir):
    """Walrus codegen allows only one sem-wait slot per engine instruction
    ("Too many sync wait commands").  Hoist all but one wait of any
    multi-wait instruction onto InstNoOp's inserted immediately before it
    on the same engine queue — semantically identical for in-order
    engines (the nop blocks the queue until its wait passes)."""
    SyncInfo = mybir.SyncInfo
    counter = [0]
    for blk in nc.m.functions[0].blocks:
        insts = list(blk.instructions)
        out, changed = [], False
        for inst in insts:
            si = getattr(inst, "sync_info", None)
            waits = list(si.on_wait) if si is not None and si.on_wait else []
            if len(waits) > 1:
                for w in waits[:-1]:
                    nop = mybir.InstNoOp(name=f"waitnop_{counter[0]}")
                    counter[0] += 1
                    nop.engine = inst.engine
                    nop.sync_info = SyncInfo(on_wait=[w], on_update=[])
                    nc.register_instruction(nop, overwrite=True)
                    out.append(nop)
                si.on_wait = [waits[-1]]
                changed = True
            out.append(inst)
        if changed:
            blk.instructions = out


def build_nc():
    import concourse.bass as bass
    import concourse.mybir as mybir
    import concourse.tile as tile

    key = 3
    if key in _NC_CACHE:
        return _NC_CACHE[key]
    nc = bass.Bass("TRN2", target_bir_lowering=False, debug=False)
    x = nc.dram_tensor("x", [CI, T * PL], mybir.dt.bfloat16,
                       kind="ExternalInput").ap()
    wt = nc.dram_tensor("wt", [128, NW * CO], mybir.dt.bfloat16,
                        kind="ExternalInput").ap()
    out = nc.dram_tensor("out", [CO, N], mybir.dt.bfloat16,
                         kind="ExternalOutput").ap()
    with tile.TileContext(nc) as tc:
        _emit(nc, tc, x, wt, out, mybir, bass)
    _dedup_ldweights(nc, mybir)
    _split_multi_waits(nc, mybir)
    _NC_CACHE[key] = nc
    return nc


def host_x(xb):
    """Pad [CI, T, R, C] f32 -> [CI, T*R*RP] bf16 with zero pad cols."""
    xp = np.zeros((CI, T, R, RP), dtype=ml_dtypes.bfloat16)
    xp[:, :, :, 0:C] = xb
    return np.ascontiguousarray(xp.reshape(CI, T * PL))


def host_weights(inputs):
    """Stack + transpose the weights into lhsT layout [i, d, o] bf16,
    ordered as DIRS, duplicated onto both partition halves [128, 7*64]."""
    names = {"self": "w_self", "tp": "w_tp", "tm": "w_tm", "rp": "w_rp",
             "rm": "w_rm", "cp": "w_cp", "cm": "w_cm"}
    ws = [np.asarray(inputs[names[d[0]]], dtype=np.float32) for d in DIRS]
    wt = np.stack([np.ascontiguousarray(w.T) for w in ws])  # [d, i, o]
    wt = wt.transpose(1, 0, 2).reshape(CI, NW * CO)         # [i, d*o]
    wt2 = np.concatenate([wt, wt], axis=0)                  # [128, d*o]
    return np.ascontiguousarray(wt2.astype(ml_dtypes.bfloat16))


def kernel(**inputs):
    from concourse.bass_utils import run_bass_kernel_spmd

    nc = build_nc()
    x = np.asarray(inputs["x"], dtype=np.float32)
    wt = host_weights(inputs)
    in_maps = [
        {"x": host_x(x[b]), "wt": wt}
        for b in range(B)
    ]
    res = run_bass_kernel_spmd(nc, in_maps, list(range(B))).results
    out = np.stack([np.asarray(res[b]["out"], dtype=np.float32)
                    .reshape(CO, T, R, C) for b in range(B)])
    return out


# revision 9
# speedup vs baseline: 3.9854x; 3.9854x over previous
"""DirectionalConv3d Trainium2 kernel (v3: full-width DMAs, streamed output).

out[b, o, t, r, c] = sum_d W_d[o, :] . x[b, :, (t,r,c)+delta_d]
for the 7-point directional stencil (self, t+-1, r+-1, c+-1), zero padded.

Strategy (per core, 1 batch per core, 8 cores):
  - Host pre-casts x to bf16 and pads each row to pitch 34 (2 zero cols), so
    c+-1 shifted reads hit zeros at row boundaries with no on-chip restaging.
  - T-halved image: partitions 0-63 hold planes 0-16, partitions 64-127 hold
    planes 15-31.  The host ships x ALREADY in this [128, 17*PL] layout so
    each image chunk is ONE plain-2D 128-partition DMA: descriptors fan out
    over all 16 SDMA engines (two 64-partition DMAs serialize at half
    bandwidth, and multi-dim DRAM APs hit a slow HWDGE descriptor path).
  - No zero halo slots: the t-boundary matmuls (tp on plane 0, tm on plane
    31) contribute exactly zero and are skipped, so no big memsets and no
    zero planes in SBUF.
  - 4 concurrent PE tile-position streams: quadrant (lhs_half*64,
    psum_half*64) computes one output plane per round; rounds r=0..7 cover
    planes (2r, 2r+1, 16+2r, 17+2r).  Each plane = 7 directional matmuls x
    2 psum chunks (out rows 0-15 / 16-31, 512 f32 = 1 bank each).  r+-1
    shifts clip the out-row range instead of padding (skipped rows == +0).
  - d-loop order alternates per round (self..cm / cm..self) so the boundary
    direction's stationaries stay loaded; LDWEIGHTS dedup drops reloads.
  - Evac round r: DVE casts ptile[h=0] [128,1024] f32->bf16 into the stage,
    ACT casts ptile[h=1]; then two plain-2D 128-partition DMAs on the scalar
    HWDGE ring (separate from the sync-ring input stream) stream the round's
    4 planes to HBM every round, overlapping the input stream.  The HBM out
    layout is [128, 16*OPL] (partition p = pb*64+ch, col = (h*8+r)*OPL+c ->
    plane t = h*16+2r+pb of channel ch); the host unshuffles it.
"""

import numpy as np
import ml_dtypes

B = 8
CI = 64
CO = 64
T = 32
R = 32
C = 32
RP = 34                  # padded row pitch (2 zero cols)
PL = R * RP              # 1088 elems per padded plane (input image)
OPL = R * C              # 1024 elems per dense output plane
N = T * OPL              # dense output elems per channel
LEAD = 2                 # zero guard elems at each image-chunk tile front
HALF = 16                # output planes per partition half
# planes per half per chunk (17 per half incl. the shared halo plane 15/16);
# small first chunks so round 0 compute starts as early as possible
CHUNK_PLANES = (2, 2, 3, 3, 3, 4)

# direction -> (dt, dr, dc); order fixed: self first (start), cm last (stop)
DIRS = [
    ("self", 0, 0, 0),
    ("rp", 0, -1, 0),
    ("rm", 0, 1, 0),
    ("cp", 0, 0, -1),
    ("tp", -1, 0, 0),
    ("tm", 1, 0, 0),
    ("cm", 0, 0, 1),
]
NW = len(DIRS)

_NC_CACHE = {}


def _chunk_of_plane(k):
    c0 = 0
    for ci, n in enumerate(CHUNK_PLANES):
        if c0 <= k < c0 + n:
            return ci, c0, n
        c0 += n
    raise AssertionError(k)


def _emit(nc, tc, x, wt, out, mybir, bass):
    bf16 = mybir.dt.bfloat16
    AP = bass.AP

    xpool = tc.alloc_tile_pool(name="xin", bufs=1)
    wpool = tc.alloc_tile_pool(name="wp", bufs=1)
    apool = tc.alloc_tile_pool(name="accp", bufs=2, space="PSUM")
    spool = tc.alloc_tile_pool(name="stg", bufs=3)

    # ---- weights: host ships [128, 7*64] (both halves pre-stacked) ----
    w_sb = wpool.tile([128, NW * CO], bf16, name="w_sb")
    nc.sync.dma_start(
        out=w_sb[0:128, :],
        in_=AP(wt.tensor, 0, [[NW * CO, 128], [1, NW * CO]]))

    # ---- image chunks: ONE plain-2D 128-partition DMA each ----
    # x HBM layout [128, 17*PL]: row p<64 = channel p planes 0..16, row
    # p>=64 = channel p-64 planes 15..31 (host pre-stacked).
    xts = []
    c0 = 0
    for ci, npl in enumerate(CHUNK_PLANES):
        # +2 tail: the rearrange view of a cm-shifted read of the last row
        # spans (but never reads) up to 2 elems past the last plane
        xt = xpool.tile([128, LEAD + npl * PL + 2], bf16, name=f"xc{ci}")
        nc.vector.memset(xt[0:128, 0:LEAD], 0.0)
        src = AP(x.tensor, c0 * PL, [[17 * PL, 128], [1, npl * PL]])
        nc.sync.dma_start(out=xt[0:128, LEAD:LEAD + npl * PL], in_=src)
        xts.append(xt)
        c0 += npl

    def w_ap(di, h):
        return w_sb[h * 64:(h + 1) * 64, di * CO:(di + 1) * CO]

    def rhs_ap(h, k, xrow0, nrows, dc):
        """rhs AP: x rows xrow0.., cols dc.. of chunk-slot k of half h."""
        ci, cs0, npl = _chunk_of_plane(k)
        xt = xts[ci]
        lo = h * 64
        start = LEAD + (k - cs0) * PL + xrow0 * RP + dc
        v = xt[lo:lo + 64, start:start + nrows * RP]
        v = v.rearrange("p (r c) -> p r c", c=RP)
        return v[:, :, 0:C]

    # ---- main loop: 8 rounds x 7 dirs x 2 row-chunks x 4 quadrants ----
    for r in range(8):
        # psum tiles: one 2-bank tile per x-half; top/bottom psum partition
        # halves hold the even/odd output plane of this round
        ptiles = {}
        for h in range(2):
            ptiles[h] = apool.tile(
                [128, 1024], mybir.dt.float32, name=f"ps{r}_{h}",
                tag=f"ps{h}")

        # alternate direction order per round: the boundary direction's
        # stationary stays loaded, saving 4 LDWEIGHTS per round boundary
        order = range(NW) if r % 2 == 0 else range(NW - 1, -1, -1)
        for kk, di in enumerate(order):
            dname, dt_, dr, dc = DIRS[di]
            first = kk == 0
            last = kk == NW - 1
            for c2 in range(2):
                row0 = c2 * 16
                # out rows valid iff 0 <= row+dr < R
                orow0 = max(row0, -dr)
                orow1 = min(row0 + 16, R - dr)
                for h in range(2):          # x-half = lhsT partition half
                    for pb in range(2):     # psum partition half
                        po = 2 * r + pb     # local out plane 0..15
                        # slot index: top plane po+dt_, bottom 15+(po+dt_+1)
                        if h == 0:
                            slot = po + dt_
                            if slot < 0:
                                continue    # tp on plane 0: zero contrib
                        else:
                            slot = po + dt_ + 1
                            if slot > 16:
                                continue    # tm on plane 31: zero contrib
                        lo = pb * 64
                        acc = ptiles[h]
                        cb = c2 * 512       # bank base inside 2-bank tile
                        if first or last:   # dr == 0: full bank
                            oap = acc[lo:lo + 64, cb:cb + 512]
                            rhs = rhs_ap(h, slot, row0, 16, dc)
                        else:
                            oap = acc[lo:lo + 64,
                                      cb + (orow0 - row0) * C:
                                      cb + (orow1 - row0) * C]
                            rhs = rhs_ap(h, slot, orow0 + dr,
                                         orow1 - orow0, dc)
                        nc.tensor.matmul(
                            out=oap, lhsT=w_ap(di, h), rhs=rhs,
                            start=first, stop=last,
                            # sim psum-group tracker aliases partition
                            # ranges within a bank; per-element on HW
                            skip_group_check=True)

        # ---- evac round r: 2 full-width cast-copies + 2 out-DMAs ----
        stage = spool.tile([128, 2 * OPL], bf16, name=f"st{r}", tag="st")
        nc.vector.tensor_copy(out=stage[:, 0:OPL], in_=ptiles[0][:, :])
        nc.scalar.copy(out=stage[:, OPL:2 * OPL], in_=ptiles[1][:, :])
        # plain-2D 128-partition DMAs into the [128, 16*OPL] staging HBM
        # layout (col (h*8+r)*OPL); they ride the scalar (ACT) HWDGE ring
        # so they never queue behind the sync-ring input stream
        for h in range(2):
            dst = AP(out.tensor, (h * 8 + r) * OPL,
                     [[HALF * OPL, 128], [1, OPL]])
            nc.scalar.dma_start(out=dst,
                                in_=stage[:, h * OPL:(h + 1) * OPL])

    for p in (spool, apool, wpool, xpool):
        p.release()


def _dedup_ldweights(nc, mybir):
    """Drop InstLdweights whose tile position already holds the same
    stationary (the PE keeps per-quadrant weights until overwritten).
    Any semaphore waits on a dropped load are preserved on an InstNoOp in
    its place on the PE queue."""
    SyncInfo = mybir.SyncInfo
    counter = [0]
    last_w = {}
    for f in nc.m.functions:
        for blk in f.blocks:
            out, changed = [], False
            for inst in blk.instructions:
                if type(inst).__name__ != "InstLdweights":
                    out.append(inst)
                    continue
                pos = tuple(inst.tile_position or (0, 0))
                wap = inst.ins[0]
                key = (wap.memref, wap.offset,
                       tuple(tuple(p) for p in wap.ap))
                si = getattr(inst, "sync_info", None)
                ups = list(si.on_update) if si is not None and si.on_update \
                    else []
                if last_w.get(pos) == key and not ups:
                    waits = list(si.on_wait) if si is not None and \
                        si.on_wait else []
                    if waits:
                        nop = mybir.InstNoOp(name=f"ldwnop_{counter[0]}")
                        counter[0] += 1
                        nop.engine = inst.engine
                        nop.sync_info = SyncInfo(on_wait=waits, on_update=[])
                        nc.register_instruction(nop, overwrite=True)
                        out.append(nop)
                    changed = True
                    continue
                last_w[pos] = key
                out.append(inst)
            if changed:
                blk.instructions = out


def _split_multi_waits(nc, mybir):
    """Walrus codegen allows only one sem-wait slot per engine instruction
    ("Too many sync wait commands").  Hoist all but one wait of any
    multi-wait instruction onto InstNoOp's inserted immediately before it
    on the same engine queue — semantically identical for in-order
    engines (the nop blocks the queue until its wait passes)."""
    SyncInfo = mybir.SyncInfo
    counter = [0]
    for blk in nc.m.functions[0].blocks:
        insts = list(blk.instructions)
        out, changed = [], False
        for inst in insts:
            si = getattr(inst, "sync_info", None)
            waits = list(si.on_wait) if si is not None and si.on_wait else []
            if len(waits) > 1:
                for w in waits[:-1]:
                    nop = mybir.InstNoOp(name=f"waitnop_{counter[0]}")
                    counter[0] += 1
                    nop.engine = inst.engine
                    nop.sync_info = SyncInfo(on_wait=[w], on_update=[])
                    nc.register_instruction(nop, overwrite=True)
                    out.append(nop)
                si.on_wait = [waits[-1]]
                changed = True
            out.append(inst)
        if changed:
            blk.instructions = out


def build_nc():
    import concourse.bass as bass
    import concourse.mybir as mybir
    import concourse.tile as tile

    key = 3
    if key in _NC_CACHE:
        return _NC_CACHE[key]
    nc = bass.Bass("TRN2", target_bir_lowering=False, debug=False)
    x = nc.dram_tensor("x", [128, 17 * PL], mybir.dt.bfloat16,
                       kind="ExternalInput").ap()
    wt = nc.dram_tensor("wt", [128, NW * CO], mybir.dt.bfloat16,
                        kind="ExternalInput").ap()
    out = nc.dram_tensor("out", [128, HALF * OPL], mybir.dt.bfloat16,
                         kind="ExternalOutput").ap()
    with tile.TileContext(nc) as tc:
        _emit(nc, tc, x, wt, out, mybir, bass)
    _dedup_ldweights(nc, mybir)
    _split_multi_waits(nc, mybir)
    _NC_CACHE[key] = nc
    return nc


def host_x(xb):
    """Pad [CI, T, R, C] f32 -> [128, 17*PL] bf16: row p<64 = channel p
    planes 0..16 (pitch RP, zero pad cols), row p>=64 = planes 15..31."""
    xp = np.zeros((CI, T, R, RP), dtype=ml_dtypes.bfloat16)
    xp[:, :, :, 0:C] = xb
    xp = xp.reshape(CI, T, PL)
    x2 = np.empty((128, 17 * PL), dtype=ml_dtypes.bfloat16)
    x2[0:64] = xp[:, 0:17].reshape(CI, 17 * PL)
    x2[64:128] = xp[:, 15:32].reshape(CI, 17 * PL)
    return x2


def host_weights(inputs):
    """Stack + transpose the weights into lhsT layout [i, d, o] bf16,
    ordered as DIRS, duplicated onto both partition halves [128, 7*64]."""
    names = {"self": "w_self", "tp": "w_tp", "tm": "w_tm", "rp": "w_rp",
             "rm": "w_rm", "cp": "w_cp", "cm": "w_cm"}
    ws = [np.asarray(inputs[names[d[0]]], dtype=np.float32) for d in DIRS]
    wt = np.stack([np.ascontiguousarray(w.T) for w in ws])  # [d, i, o]
    wt = wt.transpose(1, 0, 2).reshape(CI, NW * CO)         # [i, d*o]
    wt2 = np.concatenate([wt, wt], axis=0)                  # [128, d*o]
    return np.ascontiguousarray(wt2.astype(ml_dtypes.bfloat16))


def kernel(**inputs):
    from concourse.bass_utils import run_bass_kernel_spmd

    nc = build_nc()
    x = np.asarray(inputs["x"], dtype=np.float32)
    wt = host_weights(inputs)
    in_maps = [
        {"x": host_x(x[b]), "wt": wt}
        for b in range(B)
    ]
    res = run_bass_kernel_spmd(nc, in_maps, list(range(B))).results
    # unshuffle [128, 16*OPL]: p = pb*64+ch, col = (h*8+r)*OPL + c
    # -> plane t = h*16 + 2r + pb of channel ch
    outs = []
    for b in range(B):
        a = np.asarray(res[b]["out"], dtype=np.float32)
        a = a.reshape(2, CO, 2, 8, R, C)           # [pb, ch, h, r, rr, cc]
        a = a.transpose(1, 2, 3, 0, 4, 5)          # [ch, h, r, pb, rr, cc]
        outs.append(a.reshape(CO, T, R, C))
    return np.stack(outs)
